# revision 15
# baseline (speedup 1.0000x reference)
"""Single-head causal attention on 8 Trainium2 NeuronCores.

Sharding: core = 2*b + p handles batch b (of 4) and query tokens {2j+p}
(1024 rows, interleaved for balanced causal work). Keys stay in NATURAL
token order on every core, which makes the VWo = x@Wvo.T projection
identical across a core pair -> each core computes half (its 1024
natural tokens) and the halves are exchanged with a 2-core AllGather.

Algebra: scores = Q@K.T = x @ (Wk.T@Wq) @ x.T and (attn@V)@Wo.T =
attn @ (x@(Wo@Wv).T), so with host-precomputed G = Wk.T@Wq and
Wvo = Wo@Wv (exact fp32) the device only runs:
  QT[i,q]    = G @ xTq               (queries, own tokens)
  VWoh[l,o]  = xTa-chunks.T @ WvoT   (half of the keys) -> AllGather
  S.T[l,q]   = xTk-chunks.T @ QT     -> expT = exp(S.T/32) * causal_mask
  out[q,o]   = (expT.T @ VWo) / (expT.T @ 1)
All matmuls bf16 with fp32 PSUM accumulate. With natural key order the
causal staircase advances 2 keys per query column: score chunk cl only
needs q-columns >= 64*cl, and q-tile t only attends key chunks
0..2t+1. Output is stored bf16 (cast to fp32 host-side).
"""

import os
import sys
import types

import numpy as np
import ml_dtypes


def _ensure_ntff_hook():
    """bass_utils' trace path hard-imports antenv.axon_hooks, which the
    container's antenv stub lacks. Register the boot hook ourselves
    (best-effort) so trace=True works instead of crashing."""
    try:
        import antenv.axon_hooks  # noqa: F401
        return
    except ImportError:
        pass
    try:
        import antenv
        mod = types.ModuleType("antenv.axon_hooks")
        state = {"h": None}
        mod.set_axon_ntff_profile_hook = lambda h: state.__setitem__("h", h)
        mod.get_axon_ntff_profile_hook = lambda: state["h"]
        sys.modules["antenv.axon_hooks"] = mod
        antenv.axon_hooks = mod
        from trn_agent_boot.trn_boot import _ntff_profile_via_ctypes
        so = "/opt/axon/libaxon_pjrt.so"
        if os.path.exists(so):
            h = _ntff_profile_via_ctypes(so)
            if h is not None:
                mod.set_axon_ntff_profile_hook(h)
    except Exception:  # noqa: BLE001  # profiling is optional; never break the run
        pass


_ensure_ntff_hook()

import concourse.bass as bass
import concourse.bacc as bacc
import concourse.mybir as mybir
import concourse.tile as tile
from concourse.bass_utils import run_bass_kernel_spmd

BF16 = ml_dtypes.bfloat16
B, S, D = 4, 2048, 1024
NC = 8          # i-chunks of 128 (contraction dim)
NL = 16         # l-chunks of 128 (natural key order)

LAST_EXEC_TIME_NS = None
LAST_RESULTS = None
_CACHE = {}


def _qbs_for(cl):
    """q-blocks that consume natural score chunk cl."""
    return (0, 1) if cl < 8 else (1,)


def _is_mixed(qb, cl):
    return (qb == 0 and cl < 8) or (qb == 1 and cl >= 8)


def _off_for(qb, cl):
    """first valid q-col (64-granular) within the 512-col block frame."""
    return 64 * (cl - 8 * qb) if _is_mixed(qb, cl) else 0


def _attn_chunks(t):
    """natural key chunks attended by q-tile t."""
    return list(range(min(2 * t + 2, NL)))


def _build(with_biases: bool):
    f32, bf16 = mybir.dt.float32, mybir.dt.bfloat16
    nc = bacc.Bacc("TRN2", target_bir_lowering=False, debug=False, num_devices=8)

    # [i_loc, ic, col] chunked layouts (host-prepared, bf16)
    xTq_d = nc.dram_tensor("xTq", [128, NC, D], bf16, kind="ExternalInput")
    xTa_d = nc.dram_tensor("xTa", [128, NC, D], bf16, kind="ExternalInput")
    xTk_d = nc.dram_tensor("xTk", [128, NC, S], bf16, kind="ExternalInput")
    wg_d = nc.dram_tensor("wg", [128, NC, D], bf16, kind="ExternalInput")
    wvo_d = nc.dram_tensor("wvo", [128, NC, D], bf16, kind="ExternalInput")
    mk_d = nc.dram_tensor("mk", [128, 16, 512], bf16, kind="ExternalInput")
    if with_biases:
        vqd = nc.dram_tensor("vq", [128, NC, 1], bf16, kind="ExternalInput")
        bvod = nc.dram_tensor("bvo", [1, D], bf16, kind="ExternalInput")
    out = nc.dram_tensor("out", [1024, D], bf16, kind="ExternalOutput")
    cin = nc.dram_tensor("cin", [128, NC, D], bf16, kind="Internal")
    cout = nc.dram_tensor("cout", [2, 128, NC, D], bf16, kind="Internal")

    with tile.TileContext(nc) as tc:
        with (
            tc.tile_pool(name="big", bufs=1) as big,
            tc.tile_pool(name="cst", bufs=1) as cst,
            tc.tile_pool(name="psum", bufs=1, space=bass.MemorySpace.PSUM) as psp,
        ):
            xa_sb = big.tile([128, NC, D], bf16)     # my natural-token half
            wv_sb = big.tile([128, NC, D], bf16)
            xq_sb = big.tile([128, NC, D], bf16)     # query tokens (chunked)
            wg_sb = big.tile([128, NC, D], bf16)
            xk_sb = big.tile([128, NC, S], bf16)     # natural keys (chunked)
            mk_sb = big.tile([128, 16, 512], bf16)
            qt_sb = big.tile([128, NC, D], bf16)     # G @ xTq
            vwo_sb = big.tile([128, NL, D], bf16)    # x @ Wvo.T (natural)

            # demand-ordered input streaming (HWDGE FIFO per engine)
            nc.sync.dma_start(xa_sb[:, 0:4, :], xTa_d.ap()[:, 0:4, :])
            nc.sync.dma_start(wv_sb[:, 0:4, :], wvo_d.ap()[:, 0:4, :])
            nc.sync.dma_start(xa_sb[:, 4:8, :], xTa_d.ap()[:, 4:8, :])
            nc.sync.dma_start(wv_sb[:, 4:8, :], wvo_d.ap()[:, 4:8, :])
            nc.sync.dma_start(wg_sb[:], wg_d.ap())
            nc.sync.dma_start(xq_sb[:], xTq_d.ap())
            nc.sync.dma_start(xk_sb[:], xTk_d.ap())
            nc.sync.dma_start(mk_sb[:], mk_d.ap())

            ones_col = cst.tile([128, 1], bf16)
            nc.vector.memset(ones_col[:], 1.0)
            if with_biases:
                ones_row = cst.tile([1, 512], bf16)
                nc.vector.memset(ones_row[:], 1.0)
                vq_sb = cst.tile([128, NC, 1], bf16)
                bvo_sb = cst.tile([1, D], bf16)
                nc.sync.dma_start(vq_sb[:], vqd.ap())
                nc.sync.dma_start(bvo_sb[:], bvod.ap())
                vxl_sb = cst.tile([1, S], bf16)

            # ---- A2h: my half of VWo, staged to DRAM for the exchange ----
            pairs = [[0, 1], [2, 3], [4, 5], [6, 7]]
            for u in range(NC):
                pa0 = psp.tile([128, 512], f32, tag="paw", bufs=3)
                pa1 = psp.tile([128, 512], f32, tag="paw", bufs=3)
                for ic in range(NC):
                    lhs = xa_sb[:, ic, u * 128:(u + 1) * 128]
                    st = ic == 0
                    sp = ic == NC - 1 and not with_biases
                    nc.tensor.matmul(pa0[:], lhs, wv_sb[:, ic, 0:512], start=st, stop=sp)
                    nc.tensor.matmul(pa1[:], lhs, wv_sb[:, ic, 512:1024], start=st, stop=sp)
                if with_biases:
                    nc.tensor.matmul(pa0[:], ones_row[0:1, 0:128], bvo_sb[0:1, 0:512],
                                     start=False, stop=True)
                    nc.tensor.matmul(pa1[:], ones_row[0:1, 0:128], bvo_sb[0:1, 512:1024],
                                     start=False, stop=True)
                # stage in (any) fixed vwo slots, DMA out per chunk
                nc.vector.tensor_copy(vwo_sb[:, u, 0:512], pa0[:])
                nc.vector.tensor_copy(vwo_sb[:, u, 512:1024], pa1[:])
                nc.sync.dma_start(cin.ap()[:, u, :], vwo_sb[:, u, :])
            nc.gpsimd.collective_compute(
                "AllGather", mybir.AluOpType.bypass,
                ins=[cin.ap()], outs=[cout.ap()],
                replica_groups=pairs)

            # ---- A1: QT = G @ xTq ----
            for dc in range(NC):
                pa0 = psp.tile([128, 512], f32, tag="paw", bufs=3)
                pa1 = psp.tile([128, 512], f32, tag="paw", bufs=3)
                for ic in range(NC):
                    lw = wg_sb[:, ic, dc * 128:(dc + 1) * 128]
                    st, sp = ic == 0, ic == NC - 1
                    nc.tensor.matmul(pa0[:], lw, xq_sb[:, ic, 0:512], start=st, stop=sp)
                    nc.tensor.matmul(pa1[:], lw, xq_sb[:, ic, 512:1024], start=st, stop=sp)
                nc.scalar.copy(qt_sb[:, dc, 0:512], pa0[:])
                nc.scalar.copy(qt_sb[:, dc, 512:1024], pa1[:])

            # bias term bq.K[l]: vxl = (Wk.T bq).T @ xTk  [1, S]
            if with_biases:
                for lh in range(4):
                    pv = psp.tile([1, 512], f32, tag="pv", bufs=2)
                    for ic in range(NC):
                        nc.tensor.matmul(pv[:], vq_sb[:, ic, 0:1],
                                         xk_sb[:, ic, lh * 512:(lh + 1) * 512],
                                         start=(ic == 0), stop=(ic == NC - 1))
                    nc.vector.tensor_copy(vxl_sb[0:1, lh * 512:(lh + 1) * 512], pv[:])

            # gathered VWo (overwrites the staging slots with natural order)
            nc.sync.dma_start(vwo_sb[:, 0:8, :], cout.ap()[0])
            nc.sync.dma_start(vwo_sb[:, 8:16, :], cout.ap()[1])

            # ---- phase B: scores + attention ----
            with tc.tile_pool(name="phB", bufs=1) as phb:
                expt = {0: {}, 1: {}}
                for cl in range(NL):
                    for qb in _qbs_for(cl):
                        off = _off_for(qb, cl)
                        qs = slice(qb * 512 + off, qb * 512 + 512)
                        ps = psp.tile([128, 512], f32, tag="paw", bufs=3, name="psS")
                        for ic in range(NC):
                            st = ic == 0
                            sp = ic == NC - 1 and not with_biases
                            lhs = xk_sb[:, ic, cl * 128:(cl + 1) * 128]
                            nc.tensor.matmul(ps[:, off:512], lhs,
                                             qt_sb[:, ic, qs], start=st, stop=sp)
                        if with_biases:
                            nc.tensor.matmul(ps[:, off:512],
                                             vxl_sb[0:1, cl * 128:(cl + 1) * 128],
                                             ones_row[0:1, 0:512 - off],
                                             start=False, stop=True)
                        et = phb.tile([128, 512], bf16, tag="exp", bufs=24, name="et")
                        if off % 128:
                            # q-tile (off-64)//128 reads the 64-col sliver
                            # below off -> zero it.
                            nc.vector.memset(et[:, off - 64:off], 0.0)
                        nc.scalar.activation(et[:, off:512], ps[:, off:512],
                                             mybir.ActivationFunctionType.Exp,
                                             scale=1.0 / 32.0)
                        if _is_mixed(qb, cl):
                            m = qb * 8 + (cl - 8 * qb)
                            nc.vector.tensor_tensor(et[:, off:512], et[:, off:512],
                                                    mk_sb[:, m, off:512],
                                                    mybir.AluOpType.mult)
                        expt[qb][cl] = et

                for t in range(8):
                    qb, tl = t // 4, t % 4
                    chunks = _attn_chunks(t)
                    po0 = psp.tile([128, 512], f32, tag="po0", bufs=2)
                    po1 = psp.tile([128, 512], f32, tag="po1", bufs=2)
                    pss = psp.tile([128, 1], f32, tag="ps", bufs=1)
                    nlast = len(chunks) - 1
                    for i, cl in enumerate(chunks):
                        lw = expt[qb][cl][:, tl * 128:(tl + 1) * 128]
                        st, sp = i == 0, i == nlast
                        nc.tensor.matmul(po0[:], lw, vwo_sb[:, cl, 0:512],
                                         start=st, stop=sp)
                        nc.tensor.matmul(po1[:], lw, vwo_sb[:, cl, 512:1024],
                                         start=st, stop=sp)
                        nc.tensor.matmul(pss[:], lw, ones_col[:], start=st, stop=sp)
                    rec = phb.tile([128, 1], f32, tag="rec", bufs=4, name="rec")
                    nc.vector.reciprocal(rec[:], pss[:])
                    ot = phb.tile([128, 1024], bf16, tag="outp", bufs=3, name="ot")
                    nc.vector.tensor_scalar_mul(ot[:, 0:512], po0[:], rec[:])
                    nc.vector.tensor_scalar_mul(ot[:, 512:1024], po1[:], rec[:])
                    nc.sync.dma_start(out.ap()[t * 128:(t + 1) * 128, 0:512],
                                      ot[:, 0:512])
                    nc.sync.dma_start(out.ap()[t * 128:(t + 1) * 128, 512:1024],
                                      ot[:, 512:1024])

    nc.compile()
    return nc


def _host_weights(Wq, Wk, Wv, Wo):
    G = (Wk.T.astype(np.float64) @ Wq.astype(np.float64)).astype(np.float32)
    Wvo = (Wo.astype(np.float64) @ Wv.astype(np.float64)).astype(np.float32)

    def wlayout(W):  # lhsT/rhs layout [i_loc, ic, d] = W[d, i] i.e. W.T chunked
        return np.ascontiguousarray(
            W.T.reshape(8, 128, D).transpose(1, 0, 2)).astype(BF16)

    return wlayout(G), wlayout(Wvo)


def _chunked(xT):
    """[D, n] -> [128, 8, n] (contraction chunks of 128)."""
    return np.ascontiguousarray(
        xT.reshape(8, 128, xT.shape[1]).transpose(1, 0, 2)).astype(BF16)


def _prep_inputs(x, Wq, bq, Wk, bk, Wv, bv, Wo, bo):
    wg_a, wvo_a = _host_weights(Wq, Wk, Wv, Wo)

    i = np.arange(128)[:, None]
    jj = np.arange(512)[None, :]
    masks = {}
    for p in (0, 1):
        m = np.zeros((16, 128, 512), dtype=np.float32)
        for qb in (0, 1):
            for cl in range(16):
                if not _is_mixed(qb, cl):
                    continue
                mi = qb * 8 + (cl - 8 * qb)
                m[mi] = (128 * cl + i) <= (2 * (512 * qb + jj) + p)
        masks[p] = np.ascontiguousarray(
            m.transpose(1, 0, 2)).astype(BF16)       # [128, 16, 512]

    with_biases = _CACHE.get("with_biases", False)
    if with_biases:
        vq = (Wk.T.astype(np.float64) @ bq.astype(np.float64)).astype(np.float32)
        vq_a = np.ascontiguousarray(vq.reshape(8, 128, 1).transpose(1, 0, 2)).astype(BF16)
        bvo = (Wo.astype(np.float64) @ bv.astype(np.float64) + bo).astype(np.float32)
        bvo_a = bvo.reshape(1, D).astype(BF16)

    in_maps = []
    for core in range(8):
        b, p = core // 2, core % 2
        xT = x[b].T                                       # [D, S] natural
        im = {
            "xTq": _chunked(xT[:, p::2]),                 # my query tokens
            "xTa": _chunked(xT[:, 1024 * p:1024 * (p + 1)]),
            "xTk": _chunked(xT),
            "wg": wg_a,
            "wvo": wvo_a,
            "mk": masks[p],
        }
        if with_biases:
            im["vq"] = vq_a
            im["bvo"] = bvo_a
        in_maps.append(im)
    return in_maps


def kernel(x, Wq, bq, Wk, bk, Wv, bv, Wo, bo):
    global LAST_EXEC_TIME_NS, LAST_RESULTS
    args = [np.asarray(a, np.float32) for a in (Wq, bq, Wk, bk, Wv, bv, Wo, bo)]
    Wq, bq, Wk, bk, Wv, bv, Wo, bo = args
    # bk shifts every score of a query row equally -> cancels in softmax.
    with_biases = any(np.any(a) for a in (bq, bv, bo))
    _CACHE["with_biases"] = with_biases
    key = ("nc", with_biases)
    if key not in _CACHE:
        _CACHE[key] = _build(with_biases)
    nc = _CACHE[key]

    x = np.asarray(x, dtype=np.float32)
    in_maps = _prep_inputs(x, Wq, bq, Wk, bk, Wv, bv, Wo, bo)

    res = run_bass_kernel_spmd(nc, in_maps, list(range(8)),
                               trace=bool(os.environ.get("BASS_TRACE")))
    LAST_EXEC_TIME_NS = res.exec_time_ns
    LAST_RESULTS = res

    full = np.empty((B, S, D), dtype=np.float32)
    for core in range(8):
        b, p = core // 2, core % 2
        full[b, p::2, :] = res.results[core]["out"].astype(np.float32)
    return full


# ---------------- numpy emulation of the device program (for testing) ----
def emulate(x, Wq, bq, Wk, bk, Wv, bv, Wo, bo, cast=True):
    def cst(a):
        return a.astype(BF16).astype(np.float32) if cast else a.astype(np.float32)

    G = (Wk.T.astype(np.float64) @ Wq.astype(np.float64)).astype(np.float32)
    Wvo = (Wo.astype(np.float64) @ Wv.astype(np.float64)).astype(np.float32)
    vq = (Wk.T.astype(np.float64) @ bq.astype(np.float64)).astype(np.float32)
    bvo = (Wo.astype(np.float64) @ bv.astype(np.float64) + bo).astype(np.float32)

    full = np.empty((B, S, D), dtype=np.float32)
    i = np.arange(128)[:, None]
    jj = np.arange(512)[None, :]
    for core in range(8):
        b, p = core // 2, core % 2
        xT = cst(x[b].T)                                 # [D, S] natural
        xTq = xT[:, p::2]                                # my queries
        QT = cst(cst(G) @ xTq)                           # [D, 1024]
        VWo = cst(xT.T @ cst(Wvo).T + bvo[None, :])      # [S, D] natural
        vxl = cst(vq) @ xT                               # [S]
        outc = np.zeros((1024, D), np.float32)
        et = {0: {}, 1: {}}
        for cl in range(NL):
            for qb in _qbs_for(cl):
                off = _off_for(qb, cl)
                sc = xT[:, cl * 128:(cl + 1) * 128].T @ QT[:, qb * 512 + off:(qb + 1) * 512]
                sc = sc + vxl[cl * 128:(cl + 1) * 128][:, None]
                e = cst(np.exp(sc / 32.0))
                frame = np.zeros((128, 512), np.float32)
                frame[:, off:512] = e
                if _is_mixed(qb, cl):
                    keep = (128 * cl + i) <= (2 * (512 * qb + jj) + p)
                    frame = frame * keep
                et[qb][cl] = frame                       # [128 l, 512 q frame]
        for t in range(8):
            qb, tl = t // 4, t % 4
            num = np.zeros((128, D), np.float32)
            den = np.zeros((128, 1), np.float32)
            for cl in _attn_chunks(t):
                lw = et[qb][cl][:, tl * 128:(tl + 1) * 128]
                num += lw.T @ VWo[cl * 128:(cl + 1) * 128, :]
                den += lw.T @ np.ones((128, 1), np.float32)
            outc[t * 128:(t + 1) * 128] = cst(num / den)
        full[b, p::2, :] = outc
    return full
